# revision 65
# baseline (speedup 1.0000x reference)
"""Trainium2 Bass kernel for BaseAttention (Bahdanau-style additive attention).

Reference computation (per batch row b):
    att_h  = h @ W.T + b_h                         # [B, A]
    dot    = tanh(iaf + att_h[:, None, :])         # [B, L, A]
    scores = dot @ alpha + alpha_b                 # [B, L]
    w      = softmax(scores, axis=1)               # [B, L]
    out    = sum_l w[b, l] * af[b, l, :]           # [B, D]

Sharding: data-parallel over batch, B=128 -> 16 per core across 8 cores.

The kernel is HBM-bandwidth bound, so stream dtypes are minimized: af is
bf16 (fp8 measured at 3.1e-2 rel err vs the 2e-2 budget), while iaf and W
are fp8_e4m3 (8.8e-3 measured).  All streams are pre-tiled on the host so
every DMA lands as large contiguous per-partition descriptors.

Per-core device layout (rows = (b, l) flattened then zero-padded, RP=3200):
  - h2att as fp8 x fp8 DoubleRow matmuls (2 k-chunks each) since it runs
    before the PE p-state ramps (~20us in) at half clock.
  - rows are b-minor (r = l*16 + b): partition p always holds batch p%16,
    so att_h broadcasts to all partitions with ONE matmul (rep.T @ atthb)
    reused by every tile's add at the DVE 2x all-SBUF rate
  - scores col per tile: tanh (ACT), then the alpha-dot alternates between
    a single fused DVE op (InstTensorScalarPtr with accum_out; NOTE
    tensor_tensor_reduce faults the exec unit on this HW) and DVE-mul +
    ACT-copy-accum, so neither engine paces the chain.
  - alpha_b is dropped: a constant score shift cancels in softmax.
  - softmax denominator deferred: e = exp(scores) unnormalized; the final
    result is (sum_l e*af) * 1/(sum_l e).
  - weighted sum over l is a matmul per (tile, d-chunk) using masked lhsT
    columns: e_cols[:, b] = e * indicator(row belongs to b).  The indicator
    is zero on pad rows, which also masks them out of the denominator.
  - output shipped packed ([48, 1024] incl. garbage rows, 2 DMAs instead
    of 4) and unpacked on the host.
DMA ordering is load-bearing: w8/h_t first (gate the whole chain), iaf8
groups next, alpha/ind consts right behind the first iaf8 tile, af last.
Reordering af earlier steals queue bandwidth from the gating streams and
regresses end-to-end time.
"""

import os
from contextlib import ExitStack

import numpy as np
import ml_dtypes

import concourse.bass as bass
import concourse.mybir as mybir
import concourse.tile as tile
from concourse import bacc
from concourse.bass_utils import run_bass_kernel_spmd

F32 = mybir.dt.float32
BF16 = mybir.dt.bfloat16
FP8 = mybir.dt.float8e4
AF_T = mybir.ActivationFunctionType
NPBF16 = ml_dtypes.bfloat16
NPFP8 = ml_dtypes.float8_e4m3

B, L, D, A = 128, 196, 2048, 512
NCORES = 8
BPC = B // NCORES          # 16 batch rows per core
R = BPC * L                # 3136 valid (b, l) rows per core
P = 128                    # partitions
NT = (R + P - 1) // P      # 25 row tiles (tail zero-padded)
RP = NT * P                # 3200 padded rows
GT = 4                     # tiles per batch/DMA group: 4,4,4,4,4,4,1
KCH = D // P               # 16 k-chunks for the h @ W.T matmul
DCH = 4                    # d chunks of 512 for the weighted sum
DC = D // DCH              # 512

# small leading groups so compute starts as soon as the first tile lands;
# 1-tile trailing group keeps the post-DMA tail short
GROUPS = [(0, 1), (1, 1), (2, 2), (4, 4), (8, 4), (12, 4), (16, 4), (20, 4), (24, 1)]
# DMA granularity decoupled from compute groups.  Early DMAs kept SMALL:
# large ones steal queue bandwidth from the w8 stream that gates h2att.
IAF_DMAS = [(0, 1), (1, 1), (2, 2), (4, 4), (8, 4), (12, 4), (16, 4), (20, 5)]
AF_DMAS = [(0, 2), (2, 2), (4, 2), (6, 2), (8, 2), (10, 2), (12, 2), (14, 2),
           (16, 2), (18, 2), (20, 2), (22, 2), (24, 1)]
# (half-tile splitting of the stream tail was tried to spread the final
# bytes across queues — a single dma_start runs on one queue — but the
# extra configures measured net-worse; 2-tile groups + 1-tile tail win)
AF_TAIL = []
N_WARM = 12                # PE warm-up while DMA rings boot (p-state ramp is
                           # ~wall-clock based; this just fills dead time)


def _build_program():
    nc = bacc.Bacc(None, target_bir_lowering=False)

    h_t = nc.declare_dram_parameter("h_t", [P, KCH * BPC], FP8, isOutput=False)
    w8 = nc.declare_dram_parameter("w8", [P, KCH * A], FP8, isOutput=False)
    b_bc = nc.declare_dram_parameter("b_bc", [BPC, A], BF16, isOutput=False)
    alpha_bc = nc.declare_dram_parameter("alpha_bc", [P, A], BF16, isOutput=False)
    ind = nc.declare_dram_parameter("ind", [P, NT * BPC], FP8, isOutput=False)
    # rows are laid out b-minor (r = l*BPC + b), so partition p always holds
    # batch p%16; rep[b, p] = (p%16 == b) replicates atthb to all partitions
    # with ONE matmul instead of a broadcast matmul per tile
    rep = nc.declare_dram_parameter("rep", [BPC, P], FP8, isOutput=False)
    iaf8 = nc.declare_dram_parameter("iaf8", [P, NT * A], FP8, isOutput=False)
    aft = nc.declare_dram_parameter("aft", [P, NT * D], BF16, isOutput=False)
    # raw packed output: [48 rows, bank-major 2*DC cols]; rows 0:16 hold
    # col_grp 0, rows 32:48 col_grp 1, rows 16:32 garbage (host unpacks)
    out = nc.declare_dram_parameter("out", [48, 2 * DC], F32, isOutput=True)

    with ExitStack() as ctx:
        tc = ctx.enter_context(tile.TileContext(nc))
        consts = ctx.enter_context(tc.tile_pool(name="consts", bufs=1))
        scr = ctx.enter_context(tc.tile_pool(name="scr", bufs=3))
        ps_hb = ctx.enter_context(
            tc.tile_pool(name="ps_hb", bufs=1, space=bass.MemorySpace.PSUM)
        )
        ps_acc = ctx.enter_context(
            tc.tile_pool(name="ps_acc", bufs=1, space=bass.MemorySpace.PSUM)
        )

        # --- PE warm-up: spin the clock up out of the cold p-state while
        # the DMA rings boot; nobody reads the result ---
        warm_sb = consts.tile([P, P], BF16)
        nc.vector.memset(warm_sb[:], 0.0)
        warm_ps = ps_hb.tile([P, P], F32, tag="misc")
        for _ in range(N_WARM):
            nc.tensor.matmul(warm_ps[:], warm_sb[:], warm_sb[:], start=True, stop=True)

        # --- constants / weights (flat tiles + contiguous DMAs = one large
        # descriptor per partition; sync-ring FIFO order = priority order:
        # only what atthb/bc/tadd need precedes the iaf8 stream) ---
        WCH = 4  # w8 loaded into 4 separate tiles (tile-granular deps) so
        KPW = KCH // WCH  # h2att matmuls interleave with the w DMA chunks
        w_parts = [
            consts.tile([P, KPW * A], FP8, name=f"w{wc}", tag=f"w{wc}")
            for wc in range(WCH)
        ]
        ht_sb = consts.tile([P, KCH * BPC], FP8)
        rep_sb = consts.tile([BPC, P], FP8)
        bbc_sb = consts.tile([BPC, A], BF16)
        abc_sb = consts.tile([P, A], BF16)
        ind_sb = consts.tile([P, NT * BPC], FP8)
        nc.sync.dma_start(w_parts[0][:], w8[:, 0 : KPW * A])
        nc.sync.dma_start(ht_sb[:], h_t[:, :])
        nc.sync.dma_start(rep_sb[:], rep[:, :])
        for wc in range(1, WCH):
            sl = slice(wc * KPW * A, (wc + 1) * KPW * A)
            nc.sync.dma_start(w_parts[wc][:], w8[:, sl])
        nc.sync.dma_start(bbc_sb[:], b_bc[:, :])

        scores_all = consts.tile([P, NT], F32)
        e_all = consts.tile([P, NT], F32)
        ones_sb = consts.tile([P, 1], F32)
        nc.vector.memset(ones_sb[:], 1.0)
        eacc = consts.tile([P, GT, BPC], F32)
        nc.vector.memset(eacc[:], 0.0)

        # full-resident streams (SBUF has room; no recycling deps)
        iaf_sb = consts.tile([P, NT, A], FP8)
        aft_sb = consts.tile([P, NT, D], BF16)

        # --- att_hb = h @ W.T + b_h, shape [BPC, A].  fp8 x fp8 DoubleRow:
        # two k-chunks per matmul, halving the count on the still-cold PE ---
        atthb_ps = ps_hb.tile([BPC, A], F32, tag="misc")
        for p in range(KCH // 2):
            k = 2 * p
            nc.tensor.matmul(
                atthb_ps[:],
                ht_sb[:, k * BPC : (k + 2) * BPC].rearrange(
                    "p (t b) -> p t b", t=2
                ),
                w_parts[k // KPW][:, (k % KPW) * A : (k % KPW + 2) * A].rearrange(
                    "p (t a) -> p t a", t=2
                ),
                start=(p == 0),
                stop=(p == KCH // 2 - 1),
                perf_mode=mybir.MatmulPerfMode.DoubleRow,
            )
        atthb_sb = consts.tile([BPC, A], BF16)
        nc.vector.tensor_add(atthb_sb[:], atthb_ps[:], bbc_sb[:])

        # replicate atthb to all 128 partitions ONCE (b-minor rows: partition
        # p holds batch p%16); every tile's tadd then reads this bf16 SBUF
        # tile at the DVE 2x rate instead of a per-tile f32 PSUM broadcast
        rep_ps = ps_hb.tile([P, A], F32, tag="misc")
        nc.tensor.matmul(rep_ps[:], rep_sb[:], atthb_sb[:], start=True, stop=True)
        atthb_rep = consts.tile([P, A], BF16)
        nc.scalar.copy(atthb_rep[:], rep_ps[:])

        # --- weighted-sum accumulators: 4 d-chunks packed into TWO PSUM
        # banks at partition offsets 0/32 (matmul col_grp; offset 96 is a HW
        # bug); separate tiles per bank so the tail normalize of bank 0
        # doesn't wait on bank 1's last accumulation
        acc_b = [ps_acc.tile([64, DC], F32, name=f"acc{b}") for b in range(2)]

        # stream DMAs: iaf8 groups first (they gate the longest chain), then
        # af groups; all on the sync ring (issuing from another engine's ring
        # blocks that ring on the transfer)
        # iaf8 stream first (it gates the longest chain), af after; the tiny
        # alpha/indicator consts ride right behind the first iaf8 tile
        def dma_af(t0, n):
            # issued from the idle gpsimd ring (SWDGE): its configures run
            # in parallel with the sync ring's w8/iaf8 stream, so af bytes
            # start flowing ~8us earlier without delaying the gating streams
            nc.gpsimd.dma_start(
                aft_sb[:, t0 : t0 + n, :].rearrange("p t c -> p (t c)"),
                aft[:, t0 * D : (t0 + n) * D],
            )

        def dma_iaf(t0, n):
            nc.sync.dma_start(
                iaf_sb[:, t0 : t0 + n, :].rearrange("p t c -> p (t c)"),
                iaf8[:, t0 * A : (t0 + n) * A],
            )

        # iaf8 stream first (it gates the longest chain), af after; the tiny
        # alpha/indicator consts ride right behind the first iaf8 tile.
        # Interleaving af earlier delays the bulk configures and regresses.
        for di, (t0, n) in enumerate(IAF_DMAS):
            dma_iaf(t0, n)
            if di == 0:
                nc.sync.dma_start(abc_sb[:], alpha_bc[:, :])
                nc.sync.dma_start(ind_sb[:], ind[:, :])
        for t0, n in AF_DMAS:
            dma_af(t0, n)
        HD = D // 2
        for t, h in AF_TAIL:
            nc.sync.dma_start(
                aft_sb[:, t, h * HD : (h + 1) * HD],
                aft[:, t * D + h * HD : t * D + (h + 1) * HD],
            )

        # The weighted-sum matmuls for group g are emitted at the top of
        # group g+1 (software pipeline): this keeps PE's static order from
        # blocking group g+1's bc matmuls behind ws(g), which depends on the
        # end of group g's DVE chain.
        prev_ws = None  # (t0, n, ecols_tile)

        def emit_ws(pw):
            pt0, pn, pecols = pw
            for j in range(pn):
                for c in range(DCH):
                    ofs = 32 * (c % 2)
                    nc.tensor.matmul(
                        acc_b[c // 2][ofs : ofs + BPC, :],
                        pecols[:, j, :],
                        aft_sb[:, pt0 + j, c * DC : (c + 1) * DC],
                        start=(pt0 + j == 0),
                        stop=(pt0 + j == NT - 1),
                        skip_group_check=True,
                    )

        for gi, (t0, n) in enumerate(GROUPS):
            if prev_ws is not None:
                emit_ws(prev_ws)

            # per tile: iaf add (DVE), tanh (ACT), then ONE fused DVE op for
            # the alpha-dot: out = tanh*alpha (discarded), accum = scores col
            for j in range(n):
                tt = t0 + j
                tadd = scr.tile([P, A], BF16, tag="tadd")
                nc.vector.tensor_add(tadd[:], iaf_sb[:, tt, :], atthb_rep[:])
                tanh = scr.tile([P, A], BF16, tag="tanh")
                nc.scalar.activation(tanh[:], tadd[:], AF_T.Tanh)
                ttro = scr.tile([P, A], BF16, tag="ttro")
                # alpha-dot.  The DVE is the chain pacer, so it alternates:
                # even tiles use ONE fused DVE op (InstTensorScalarPtr:
                # out = tanh*alpha discarded, accum = sum -> scores col;
                # tensor_tensor_reduce hits a HW exec-unit fault), odd tiles
                # split DVE-mul + ACT-copy-accum to balance the engines.
                if tt % 2 == 0:
                    nc.vector.scalar_tensor_tensor(
                        out=ttro[:],
                        in0=tanh[:],
                        scalar=1.0,
                        in1=abc_sb[:],
                        op0=mybir.AluOpType.mult,
                        op1=mybir.AluOpType.mult,
                        accum_out=scores_all[:, tt : tt + 1],
                    )
                else:
                    nc.vector.tensor_mul(ttro[:], tanh[:], abc_sb[:])
                    dummy = scr.tile([P, A], BF16, tag="dummy")
                    nc.scalar.activation(
                        dummy[:],
                        ttro[:],
                        AF_T.Copy,
                        accum_out=scores_all[:, tt : tt + 1],
                    )

            # alpha_b would shift every score equally; softmax cancels a
            # constant shift, so the bias is dropped entirely
            nc.scalar.activation(
                e_all[:, t0 : t0 + n],
                scores_all[:, t0 : t0 + n],
                AF_T.Exp,
            )

            # masked weight columns: e_cols[:, j, b] = e * (row belongs to b)
            ecols = scr.tile([P, GT, BPC], BF16, tag="ecols")
            nc.vector.tensor_mul(
                ecols[:, :n, :],
                ind_sb[:, t0 * BPC : (t0 + n) * BPC].rearrange(
                    "p (t b) -> p t b", t=n
                ),
                e_all[:, t0 : t0 + n].unsqueeze(2).broadcast_to([P, n, BPC]),
            )
            nc.vector.tensor_add(eacc[:, :n, :], eacc[:, :n, :], ecols[:, :n, :])
            prev_ws = (t0, n, ecols)

        emit_ws(prev_ws)

        # --- softmax denominator (replicated to the col_grp partition
        # offsets so normalization aligns), normalize, store ---
        sums_ps = ps_hb.tile([64, 1], F32, tag="misc")
        for c in range(2):
            for j in range(GT):
                nc.tensor.matmul(
                    sums_ps[32 * c : 32 * c + BPC, :],
                    eacc[:, j, :],
                    ones_sb[:],
                    start=(j == 0),
                    stop=(j == GT - 1),
                    skip_group_check=True,
                )
        # ONE reciprocal over partitions 0..47 covers both col_grp offsets
        # (16..31 is garbage, never read); then one normalize per bank —
        # DVE takes bank 0, ACT bank 1, running in parallel.  Each bank is
        # shipped with a single [48, DC] DMA (garbage rows included) and the
        # host unpacks rows 0:16 / 32:48 — halves the serial HWDGE configures
        recip = consts.tile([64, 1], F32)
        outs_sb = [consts.tile([64, DC], F32, name=f"osb{b}") for b in range(2)]
        nc.vector.reciprocal(recip[0:48, :], sums_ps[0:48, :])
        nc.vector.tensor_scalar_mul(
            outs_sb[0][0:48, :], acc_b[0][0:48, :], recip[0:48, :]
        )
        nc.scalar.mul(outs_sb[1][0:48, :], acc_b[1][0:48, :], recip[0:48, :])
        for b in range(2):
            nc.sync.dma_start(out[:, b * DC : (b + 1) * DC], outs_sb[b][0:48, :])

    nc.compile()
    return nc


_PROGRAM = None


def _get_program():
    global _PROGRAM
    if _PROGRAM is None:
        _PROGRAM = _build_program()
    return _PROGRAM


def _host_prep(h, att_feats, internal_att_feats, h2att_w, h2att_b, alpha_w, alpha_b):
    h = np.asarray(h, np.float32)
    att_feats = np.asarray(att_feats, np.float32)
    iaf = np.asarray(internal_att_feats, np.float32)
    h2att_w = np.asarray(h2att_w, np.float32)
    h2att_b = np.asarray(h2att_b, np.float32)
    alpha_w = np.asarray(alpha_w, np.float32)
    alpha_b = np.asarray(alpha_b, np.float32)

    # W.T [D, A] pre-tiled to [P, KCH*A]: w8[p, k*A+a] = W[a, k*128+p]
    w8 = np.ascontiguousarray(
        h2att_w.T.reshape(KCH, P, A).transpose(1, 0, 2).reshape(P, KCH * A)
    ).astype(NPFP8)
    b_bc = np.tile(h2att_b.reshape(1, A), (BPC, 1)).astype(NPBF16)
    alpha_bc = np.tile(alpha_w.reshape(1, A), (P, 1)).astype(NPBF16)

    # rows are b-minor: r = l*BPC + b, so partition p holds batch p % BPC.
    # row -> batch indicator over the padded row space (0 on pad rows)
    rows = np.arange(RP)
    onehot = np.zeros((RP, BPC), np.float32)
    valid = rows < R
    onehot[rows[valid], rows[valid] % BPC] = 1.0
    ind_arr = (
        onehot.reshape(NT, P, BPC).transpose(1, 0, 2).reshape(P, NT * BPC)
    ).astype(NPFP8)
    rep_arr = np.zeros((BPC, P), np.float32)
    rep_arr[np.arange(P) % BPC, np.arange(P)] = 1.0
    rep_arr = rep_arr.astype(NPFP8)

    in_maps = []
    for i in range(NCORES):
        sl = slice(i * BPC, (i + 1) * BPC)
        h_t = np.ascontiguousarray(
            h[sl].T.reshape(KCH, P, BPC).transpose(1, 0, 2).reshape(P, KCH * BPC)
        ).astype(NPFP8)

        iaf_rows = np.zeros((RP, A), np.float32)
        iaf_rows[:R] = iaf[sl].transpose(1, 0, 2).reshape(R, A)
        af_rows = np.zeros((RP, D), np.float32)
        af_rows[:R] = att_feats[sl].transpose(1, 0, 2).reshape(R, D)
        iaf8_arr = np.ascontiguousarray(
            iaf_rows.reshape(NT, P, A).transpose(1, 0, 2)
        ).astype(NPFP8)
        aft_arr = np.ascontiguousarray(
            af_rows.reshape(NT, P, D).transpose(1, 0, 2)
        ).astype(NPBF16)

        in_maps.append(
            {
                "h_t": h_t,
                "w8": w8,
                "b_bc": b_bc,
                "alpha_bc": alpha_bc,
                "ind": ind_arr,
                "rep": rep_arr,
                "iaf8": iaf8_arr.reshape(P, NT * A),
                "aft": aft_arr.reshape(P, NT * D),
            }
        )
    return in_maps


def run(trace=False, **inputs):
    """Run the SPMD kernel; returns (full_output [B, D], BassKernelResults)."""
    nc = _get_program()
    in_maps = _host_prep(**inputs)
    res = run_bass_kernel_spmd(nc, in_maps, list(range(NCORES)), trace=trace)
    # unpack [48, 2*DC] (col_grp 0 at rows 0:16, col_grp 1 at rows 32:48,
    # bank-major columns) -> [BPC, D]
    outs = []
    for i in range(NCORES):
        o = res.results[i]["out"]
        full = np.empty((BPC, D), np.float32)
        for c in range(DCH):
            rows = slice(32 * (c % 2), 32 * (c % 2) + BPC)
            cols = slice((c // 2) * DC, (c // 2 + 1) * DC)
            full[:, c * DC : (c + 1) * DC] = o[rows, cols]
        outs.append(full)
    out = np.concatenate(outs, axis=0)
    return out, res


def kernel(**inputs):
    out, _ = run(trace=False, **inputs)
    return out
